# revision 4
# baseline (speedup 1.0000x reference)
"""LoRA linear kernel for Trainium2 (8 NeuronCores, SPMD data-parallel).

Computes out = x @ (A @ B) for
    x: [4, 2048, 4096] f32, A: [4096, 16] f32, B: [16, 4096] f32
by reassociating to (x @ A) @ B  (4.3 GFLOP instead of 274 GFLOP).

Sharding: x is split row-wise (batch*seq = 8192 rows -> 1024 rows/core).
A and B are replicated to every core. No collectives.

All matmul operands are fp16 (1 cycle/row on the PE array vs 4 for
fp32) and the output is shipped back as fp16 and upcast on the host,
halving HBM traffic in both directions. PSUM accumulation stays fp32.
Host-side prep lays x out as xTb[p, rc, c, n] so each row-chunk's DMA
is a single per-partition-contiguous line.

The R=16 contraction/output dims would leave most of the PE array
idle, so NWAY row-blocks are processed concurrently in disjoint
32-wide strips of the array via tile_position:
  stage 1 (col strips): strip g computes tT_g[16,128] = (x_blk_g @ A).T
      accumulating into PSUM partitions 32g..32g+16.
  stage 2 (row strips): strip g computes out_blk_g[128, dc] =
      tT_g.T @ B from SBUF partitions 32g..32g+16 (B replicated there).

The row loop uses NCH=4 chunks of 256 rows for a fine-grained pipeline:
chunk k's input DMA overlaps chunk k-1's compute and chunk k-2's
output DMA, keeping the HBM port saturated end to end.
"""

import numpy as np

import concourse.bass as bass
import concourse.bacc as bacc
import concourse.mybir as mybir
from concourse.tile import TileContext
from concourse.bass_utils import run_bass_kernel_spmd

N_CORES = 8
BATCH, SEQ, D_IN, D_OUT, R = 4, 2048, 4096, 4096, 16
ROWS = BATCH * SEQ              # 8192
RPC = ROWS // N_CORES           # 1024 rows per core
KC = D_IN // 128                # 32 contraction chunks of 128
DC = 512                        # d_out columns per stage-2 matmul (PSUM bank)
NDC = D_OUT // DC               # 8

F32 = mybir.dt.float32
F16 = mybir.dt.float16

NWAY = 2                        # concurrent 128-row blocks (PE strips)
RCHUNK = 128 * NWAY             # 256 rows per chunk
NCH = RPC // RCHUNK             # 4 chunks per core

_cache = {}


def _build(mm_dtype=F16):
    nc = bacc.Bacc("TRN2", target_bir_lowering=False)
    # xTb[p, rc, c, n] = x_shard[rc*RCHUNK + n, c*128 + p]
    xTb = nc.dram_tensor("xTb", [128, NCH, KC, RCHUNK], mm_dtype,
                         kind="ExternalInput")
    A = nc.dram_tensor("A", [D_IN, R], mm_dtype, kind="ExternalInput")
    Bw = nc.dram_tensor("Bw", [R, D_OUT], mm_dtype, kind="ExternalInput")
    out = nc.dram_tensor("out", [RPC, D_OUT], mm_dtype,
                         kind="ExternalOutput")

    A3 = A.rearrange("(c p) r -> p c r", p=128)     # [128, KC, R]

    with TileContext(nc) as tc:
        with (
            tc.tile_pool(name="consts", bufs=1) as cpool,
            tc.tile_pool(name="xin", bufs=3) as xpool,
            tc.tile_pool(name="tbuf", bufs=2) as tpool,
            tc.tile_pool(name="obuf", bufs=2 * NWAY) as opool,
            tc.tile_pool(name="pt", bufs=2, space="PSUM") as ptpool,
            tc.tile_pool(name="po", bufs=6, space="PSUM") as popool,
        ):
            a_tile = cpool.tile([128, KC, R], mm_dtype)
            nc.sync.dma_start(out=a_tile[:], in_=A3[:, :, :])
            # first x chunk before B so the critical path starts early
            xts = [None] * NCH
            xts[0] = xpool.tile([128, KC, RCHUNK], mm_dtype,
                                name="xt", tag="xt")
            nc.sync.dma_start(out=xts[0][:], in_=xTb[:, 0, :, :])
            # B replicated into partition strips 32g..32g+16
            b4 = cpool.tile([128, D_OUT], mm_dtype)
            for g in range(NWAY):
                nc.sync.dma_start(out=b4[32 * g:32 * g + R, :], in_=Bw[:, :])

            for rc in range(NCH):
                n0 = rc * RCHUNK
                if xts[rc] is None:
                    xts[rc] = xpool.tile([128, KC, RCHUNK], mm_dtype,
                                         name="xt", tag="xt")
                    nc.sync.dma_start(out=xts[rc][:], in_=xTb[:, rc, :, :])
                xt = xts[rc]

                # stage 1: NWAY concurrent col-strip matmuls; strip g
                # accumulates tT of row-block g into psum partitions
                # 32g..32g+16.
                pt = ptpool.tile([128, 128], F32)
                for c in range(KC):
                    for g in range(NWAY):
                        nc.tensor.matmul(
                            pt[32 * g:32 * g + R, :],
                            a_tile[:, c, :],
                            xt[:, c, 128 * g:128 * (g + 1)],
                            start=(c == 0),
                            stop=(c == KC - 1),
                            tile_position=(0, 32 * g),
                            skip_group_check=True,
                        )
                tT4 = tpool.tile([128, 128], mm_dtype)
                nc.vector.tensor_copy(tT4[:], pt[:])

                # stage 2: NWAY concurrent row-strip matmuls per dc
                osbs = [opool.tile([128, D_OUT], mm_dtype, name=f"osb{g}",
                                   tag="osb")
                        for g in range(NWAY)]
                for dc in range(NDC):
                    for g in range(NWAY):
                        po = popool.tile([128, DC], F32, name=f"po{g}",
                                         tag="po")
                        nc.tensor.matmul(
                            po[:],
                            tT4[32 * g:32 * g + R, :],
                            b4[32 * g:32 * g + R, dc * DC:(dc + 1) * DC],
                            start=True,
                            stop=True,
                            tile_position=(32 * g, 0),
                            skip_group_check=True,
                        )
                        # Split PSUM evacuation between DVE and ACT
                        if (dc * NWAY + g) % 2 == 0:
                            nc.vector.tensor_copy(
                                osbs[g][:, dc * DC:(dc + 1) * DC], po[:])
                        else:
                            nc.scalar.copy(
                                out=osbs[g][:, dc * DC:(dc + 1) * DC],
                                in_=po[:])
                for g in range(NWAY):
                    row0 = n0 + 128 * g
                    nc.sync.dma_start(out=out[row0:row0 + 128, :],
                                      in_=osbs[g][:])
    nc.compile()
    return nc


def _get_nc(mm_dtype=F16):
    key = (str(mm_dtype),)
    if key not in _cache:
        _cache[key] = _build(mm_dtype)
    return _cache[key]


def kernel(x, A, B, trace=False, mm_dtype=None):
    if mm_dtype is None:
        mm_dtype = F16
    x = np.asarray(x, dtype=np.float32)
    Ah = np.ascontiguousarray(np.asarray(A)).astype(np.float16)
    Bh = np.ascontiguousarray(np.asarray(B)).astype(np.float16)
    xf = x.reshape(ROWS, D_IN)

    nc = _get_nc(mm_dtype)
    in_maps = []
    for i in range(N_CORES):
        xs = xf[i * RPC:(i + 1) * RPC]                 # [1024, 4096]
        # xTb[p, rc, c, n] = xs[rc*RCHUNK+n, c*128+p]
        xTb = np.ascontiguousarray(
            xs.reshape(NCH, RCHUNK, KC, 128).transpose(3, 0, 2, 1)
        ).astype(np.float16)
        in_maps.append({"xTb": xTb, "A": Ah, "Bw": Bh})

    res = run_bass_kernel_spmd(nc, in_maps, list(range(N_CORES)), trace=trace)
    outs = [res.results[i]["out"] for i in range(N_CORES)]
    full = np.concatenate(outs, axis=0).astype(np.float32)
    full = full.reshape(BATCH, SEQ, D_OUT)
    if trace:
        kernel.last_exec_time_ns = res.exec_time_ns
        kernel.last_results = res
    return full


# revision 6
# speedup vs baseline: 1.0807x; 1.0807x over previous
"""LoRA linear kernel for Trainium2 (8 NeuronCores, SPMD data-parallel).

Computes out = x @ (A @ B) for
    x: [4, 2048, 4096] f32, A: [4096, 16] f32, B: [16, 4096] f32
by reassociating to (x @ A) @ B  (4.3 GFLOP instead of 274 GFLOP).

Sharding: x is split row-wise (batch*seq = 8192 rows -> 1024 rows/core).
A and B are replicated to every core. No collectives.

All matmul operands are fp16 (1 cycle/row on the PE array vs 4 for
fp32) and the output is shipped back as fp16 and upcast on the host,
halving HBM traffic in both directions. PSUM accumulation stays fp32.
Host-side prep lays x out as xTb[p, rc, h, c, n] so every input DMA is
per-partition contiguous with >=512B lines.

The R=16 contraction/output dims would leave most of the PE array
idle, so NWAY row-blocks are processed concurrently in disjoint
32-wide strips of the array via tile_position:
  stage 1 (col strips): strip g computes tT_g[16,128] = (x_blk_g @ A).T
      accumulating into PSUM partitions 32g..32g+16.
  stage 2 (row strips): strip g computes out_blk_g[128, dc] =
      tT_g.T @ B from SBUF partitions 32g..32g+16 (B replicated there).

ALL input DMAs are issued up front (SBUF holds the whole 8 MiB shard)
so the Sync engine's in-order stream never gates an input transfer on
a compute-dependent output trigger; the HBM port streams the input at
line rate from t=0 while compute and output DMAs pipeline behind it.
"""

import numpy as np

import concourse.bass as bass
import concourse.bacc as bacc
import concourse.mybir as mybir
from concourse.tile import TileContext
from concourse.bass_utils import run_bass_kernel_spmd

N_CORES = 8
BATCH, SEQ, D_IN, D_OUT, R = 4, 2048, 4096, 4096, 16
ROWS = BATCH * SEQ              # 8192
RPC = ROWS // N_CORES           # 1024 rows per core
KC = D_IN // 128                # 32 contraction chunks of 128
DC = 512                        # d_out columns per stage-2 matmul (PSUM bank)
NDC = D_OUT // DC               # 8

F32 = mybir.dt.float32
F16 = mybir.dt.float16

NWAY = 2                        # concurrent 128-row blocks (PE strips)
RCHUNK = 128 * NWAY             # 256 rows per chunk
NCH = RPC // RCHUNK             # 4 chunks per core
NSPLIT = 2                      # input DMA pieces per chunk (split over KC)

_cache = {}


def _build(mm_dtype=F16):
    nc = bacc.Bacc("TRN2", target_bir_lowering=False)
    kcs = KC // NSPLIT
    # xTb[p, rc, h, c, n] = x_shard[rc*RCHUNK + n, (h*kcs + c)*128 + p]
    xTb = nc.dram_tensor("xTb", [128, NCH, NSPLIT, kcs, RCHUNK], mm_dtype,
                         kind="ExternalInput")
    A = nc.dram_tensor("A", [D_IN, R], mm_dtype, kind="ExternalInput")
    Bw = nc.dram_tensor("Bw", [R, D_OUT], mm_dtype, kind="ExternalInput")
    out = nc.dram_tensor("out", [RPC, D_OUT], mm_dtype,
                         kind="ExternalOutput")

    A3 = A.rearrange("(c p) r -> p c r", p=128)     # [128, KC, R]

    with TileContext(nc) as tc:
        with (
            tc.tile_pool(name="consts", bufs=1) as cpool,
            tc.tile_pool(name="xin", bufs=NCH * NSPLIT) as xpool,
            tc.tile_pool(name="tbuf", bufs=2) as tpool,
            tc.tile_pool(name="obuf", bufs=2 * NWAY) as opool,
            tc.tile_pool(name="pt", bufs=2, space="PSUM") as ptpool,
            tc.tile_pool(name="po", bufs=6, space="PSUM") as popool,
        ):
            a_tile = cpool.tile([128, KC, R], mm_dtype)
            nc.sync.dma_start(out=a_tile[:], in_=A3[:, :, :])
            # the entire input shard, issued up front
            xts = {}
            for rc in range(NCH):
                for h in range(NSPLIT):
                    xt = xpool.tile([128, kcs, RCHUNK], mm_dtype,
                                    name="xt", tag="xt")
                    nc.sync.dma_start(out=xt[:], in_=xTb[:, rc, h, :, :])
                    xts[rc, h] = xt
            # B replicated into partition strips 32g..32g+16
            b4 = cpool.tile([128, D_OUT], mm_dtype)
            for g in range(NWAY):
                nc.sync.dma_start(out=b4[32 * g:32 * g + R, :], in_=Bw[:, :])

            for rc in range(NCH):
                n0 = rc * RCHUNK

                # stage 1: NWAY concurrent col-strip matmuls; strip g
                # accumulates tT of row-block g into psum partitions
                # 32g..32g+16.
                pt = ptpool.tile([128, 128], F32)
                for h in range(NSPLIT):
                    xt = xts[rc, h]
                    for c in range(kcs):
                        for g in range(NWAY):
                            nc.tensor.matmul(
                                pt[32 * g:32 * g + R, :],
                                a_tile[:, h * kcs + c, :],
                                xt[:, c, 128 * g:128 * (g + 1)],
                                start=(h == 0 and c == 0),
                                stop=(h == NSPLIT - 1 and c == kcs - 1),
                                tile_position=(0, 32 * g),
                                skip_group_check=True,
                            )
                tT4 = tpool.tile([128, 128], mm_dtype)
                nc.vector.tensor_copy(tT4[:], pt[:])

                # stage 2: NWAY concurrent row-strip matmuls per dc
                osbs = [opool.tile([128, D_OUT], mm_dtype, name=f"osb{g}",
                                   tag="osb")
                        for g in range(NWAY)]
                for dc in range(NDC):
                    for g in range(NWAY):
                        po = popool.tile([128, DC], F32, name=f"po{g}",
                                         tag="po")
                        nc.tensor.matmul(
                            po[:],
                            tT4[32 * g:32 * g + R, :],
                            b4[32 * g:32 * g + R, dc * DC:(dc + 1) * DC],
                            start=True,
                            stop=True,
                            tile_position=(32 * g, 0),
                            skip_group_check=True,
                        )
                        # Split PSUM evacuation between DVE and ACT
                        if (dc * NWAY + g) % 2 == 0:
                            nc.vector.tensor_copy(
                                osbs[g][:, dc * DC:(dc + 1) * DC], po[:])
                        else:
                            nc.scalar.copy(
                                out=osbs[g][:, dc * DC:(dc + 1) * DC],
                                in_=po[:])
                for g in range(NWAY):
                    row0 = n0 + 128 * g
                    nc.sync.dma_start(out=out[row0:row0 + 128, :],
                                      in_=osbs[g][:])
    nc.compile()
    return nc


def _get_nc(mm_dtype=F16):
    key = (str(mm_dtype),)
    if key not in _cache:
        _cache[key] = _build(mm_dtype)
    return _cache[key]


def kernel(x, A, B, trace=False, mm_dtype=None):
    if mm_dtype is None:
        mm_dtype = F16
    x = np.asarray(x, dtype=np.float32)
    Ah = np.ascontiguousarray(np.asarray(A)).astype(np.float16)
    Bh = np.ascontiguousarray(np.asarray(B)).astype(np.float16)
    xf = x.reshape(ROWS, D_IN)

    nc = _get_nc(mm_dtype)
    in_maps = []
    for i in range(N_CORES):
        xs = xf[i * RPC:(i + 1) * RPC]                 # [1024, 4096]
        # xTb[p, rc, h*kcs+c, n] = xs[rc*RCHUNK+n, (h*kcs+c)*128+p]
        xTb = np.ascontiguousarray(
            xs.reshape(NCH, RCHUNK, KC, 128).transpose(3, 0, 2, 1)
        ).astype(np.float16).reshape(128, NCH, NSPLIT, KC // NSPLIT, RCHUNK)
        in_maps.append({"xTb": xTb, "A": Ah, "Bw": Bh})

    res = run_bass_kernel_spmd(nc, in_maps, list(range(N_CORES)), trace=trace)
    outs = [res.results[i]["out"] for i in range(N_CORES)]
    full = np.concatenate(outs, axis=0).astype(np.float32)
    full = full.reshape(BATCH, SEQ, D_OUT)
    if trace:
        kernel.last_exec_time_ns = res.exec_time_ns
        kernel.last_results = res
    return full
